# revision 1
# baseline (speedup 1.0000x reference)
"""Causal self-attention (B=4, T=2048, C=1024, NH=16) on 8 TRN2 NeuronCores.

Sharding (tensor-parallel heads x data-parallel batch):
  - 4 core-pairs: pair p = cores (2p, 2p+1) handles batch b = p.
  - Within a pair, rank 0 computes heads 0-7, rank 1 heads 8-15
    (w_qkv output columns split by head group).
  - After attention each core holds attnT [512, T] (d-major, its 8 heads).
    A pairwise AllGather exchanges these; each core then computes a
    512-column half of the output projection (w_proj column split), so no
    all-reduce is needed. Host concatenates the halves.

Device algorithm (per core):
  Phase 1  QKV: xT (c-major x, transposed on host) @ w slices on the PE.
           q/k are produced d-major ([c_out, t]); v t-major with a fused
           ones-column for softmax denominators. q is scaled by 1/8 and
           spilled to HBM (SBUF pressure), k/v stay resident.
  Phase 2  Attention per (head-pair j, 512-wide q block):
           sT[kv,q] = kT_chunk.T @ qT (K=64, two heads packed in PE row
           groups 0-63/64-127), exp on ScalarE (no max subtraction: scores
           are ~N(0,1) so exp cannot overflow), causal mask = one static
           128x128 triangle multiply on the diagonal chunk, then
           aug[65,q] += v_aug.T @ aT accumulated over kv chunks in PSUM.
           Row 64 of aug is the softmax denominator (ones column of v_aug);
           normalize via DVE reciprocal + GpSimd partition-broadcast + mul.
  Phase 3  Pairwise AllGather of attnT blocks (issued per head-pair so they
           overlap remaining attention), then y[t,512] = attnT.T @ w_proj
           half, accumulated over 8 c_in blocks of 128.

All matmuls run in float32r (TF32-like, full PE rate at free-dim >= 256,
measured ~1.5e-4 max rel err per matmul): every matmul-operand tile and its
DMA source is declared float32r (same bytes as fp32 host-side); PSUM stays
fp32.
"""

import numpy as np

import concourse.bass as bass
import concourse.mybir as mybir
import concourse.tile as tile
from concourse import bacc
from concourse.bass_utils import run_bass_kernel_spmd

B, T, C = 4, 2048, 1024
NH, HD = 16, 64
N_CORES = 8
HPC = NH // 2          # heads per core
NPAIR = HPC // 2       # head-pairs per core (PE row-group packing unit)
TB = T // 128          # 128-row t blocks
QBS = T // 512         # 512-wide q blocks
KC = C // 128          # 128-deep contraction chunks for qkv/proj
SCALE = float(1.0 / np.sqrt(HD))

F32 = mybir.dt.float32
F32R = mybir.dt.float32r
AF = mybir.ActivationFunctionType
REPLICA_GROUPS = [[0, 1], [2, 3], [4, 5], [6, 7]]


def build_nc(reps=1, single_core=False):
    nc = bacc.Bacc(
        "TRN2", target_bir_lowering=False, debug=False,
        num_devices=(1 if single_core else N_CORES),
    )

    xt = nc.dram_tensor("xt", [C, T], F32R, kind="ExternalInput")
    wq = nc.dram_tensor("wq", [C, 512], F32R, kind="ExternalInput")
    wk = nc.dram_tensor("wk", [C, 512], F32R, kind="ExternalInput")
    wv = nc.dram_tensor("wv", [C, 512], F32R, kind="ExternalInput")
    wp = nc.dram_tensor("wp", [C, 512], F32R, kind="ExternalInput")
    mask = nc.dram_tensor("mask", [128, 128], F32R, kind="ExternalInput")
    y = nc.dram_tensor("y", [T, 512], F32, kind="ExternalOutput")

    with tile.TileContext(nc) as tc:
        for _rep in range(reps):
            _emit_one(nc, tc, xt, wq, wk, wv, wp, mask, y, single_core)

    nc.compile()
    return nc


def _emit_one(nc, tc, xt, wq, wk, wv, wp, mask, y, single_core):
    with tc.tile_pool(name="qspilld", bufs=1, space="DRAM") as qspd, \
         tc.tile_pool(name="persist", bufs=1) as persist, \
         tc.tile_pool(name="ccin", bufs=NPAIR, space="DRAM") as ccinp, \
         tc.tile_pool(name="ccout", bufs=NPAIR, space="DRAM") as ccoutp, \
         tc.tile_pool(name="qt", bufs=2) as qtp, \
         tc.tile_pool(name="aT", bufs=3) as atp, \
         tc.tile_pool(name="norm", bufs=2) as normp, \
         tc.tile_pool(name="nb", bufs=2) as nbp, \
         tc.tile_pool(name="ps_s", bufs=2, space="PSUM") as pss, \
         tc.tile_pool(name="augA", bufs=2, space="PSUM") as augAp, \
         tc.tile_pool(name="augB", bufs=2, space="PSUM") as augBp:
        qspill = qspd.tile([NPAIR, 128, T], F32R)
        kT_sb = persist.tile([128, NPAIR, T], F32R)
        v_sb = persist.tile([128, TB, HPC, HD + 1], F32R)
        mask_sb = persist.tile([128, 128], F32R)
        nc.sync.dma_start(out=mask_sb[:], in_=mask[:])
        wp_sb = persist.tile([128, KC, 512], F32R)
        wpr = wp[:].rearrange("(a p) n -> p a n", p=128)

        cc_outs = []
        with tc.tile_pool(name="xtp", bufs=1) as xtp, \
             tc.tile_pool(name="wvp", bufs=1) as wvp, \
             tc.tile_pool(name="wqk", bufs=1) as wqkp, \
             tc.tile_pool(name="qsp", bufs=2) as qspp:
            ones_sc = qspp.tile([128, TB * HPC], F32, tag="ones")
            nc.vector.memset(ones_sc[:], 1.0)
            nc.vector.tensor_copy(
                v_sb[:, :, :, HD],
                ones_sc[:].rearrange("p (a b) -> p a b", a=TB),
            )
            xt_sb = xtp.tile([128, KC, T], F32R)
            xt_r = xt[:].rearrange("(a p) t -> p a t", p=128)
            wv_sb = wvp.tile([128, KC, 512], F32R)
            wv_r = wv[:].rearrange("(a p) n -> p a n", p=128)
            for a in range(KC):
                if a == 0:
                    nc.sync.dma_start(
                        out=xt_sb[:, 0, 0:1024], in_=xt_r[:, 0, 0:1024])
                    nc.sync.dma_start(out=wv_sb[:, 0, :], in_=wv_r[:, 0, :])
                    nc.sync.dma_start(
                        out=xt_sb[:, 0, 1024:T], in_=xt_r[:, 0, 1024:T])
                else:
                    nc.sync.dma_start(out=xt_sb[:, a, :], in_=xt_r[:, a, :])
                    nc.sync.dma_start(out=wv_sb[:, a, :], in_=wv_r[:, a, :])
            nc.sync.dma_start(out=wp_sb[:], in_=wpr[:])
            wq_r = wq[:].rearrange("(a p) n -> p a n", p=128)
            wk_r = wk[:].rearrange("(a p) n -> p a n", p=128)

            # v: out[t 128, c_out 512] = xT_chunk.T @ w_v  (t-block major)
            for tb in range(TB):
                ps = pss.tile([128, 1024], F32, tag="s2", name="psv")
                for a in range(KC):
                    nc.tensor.matmul(
                        ps[:, 0:512],
                        xt_sb[:, a, 128 * tb:128 * (tb + 1)],
                        wv_sb[:, a, :],
                        start=(a == 0), stop=(a == KC - 1),
                    )
                nc.vector.tensor_copy(
                    v_sb[:, tb, :, 0:HD],
                    ps[:, 0:512].rearrange("p (h d) -> p h d", h=HPC),
                )

            for j in range(NPAIR):
                # q(j), k(j): out[c_out 128, t 512] = w_block.T @ xT
                wqj = wqkp.tile([128, KC, 128], F32R, tag="wqj")
                wkj = wqkp.tile([128, KC, 128], F32R, tag="wkj")
                nc.sync.dma_start(
                    out=wqj[:], in_=wq_r[:, :, 128 * j:128 * (j + 1)])
                nc.sync.dma_start(
                    out=wkj[:], in_=wk_r[:, :, 128 * j:128 * (j + 1)])
                for which in ("q", "k"):
                    wsb = wqj if which == "q" else wkj
                    for i in range(QBS):
                        ps = pss.tile([128, 1024], F32, tag="s2", name="psqk")
                        for a in range(KC):
                            nc.tensor.matmul(
                                ps[:, 0:512],
                                wsb[:, a, :],
                                xt_sb[:, a, 512 * i:512 * (i + 1)],
                                start=(a == 0), stop=(a == KC - 1),
                            )
                        if which == "q":
                            sp = qspp.tile([128, 512], F32R, tag="qsp")
                            nc.vector.tensor_scalar_mul(
                                sp[:], ps[:, 0:512], SCALE)
                            nc.sync.dma_start(
                                out=qspill[j, :, 512 * i:512 * (i + 1)],
                                in_=sp[:],
                            )
                        else:
                            nc.vector.tensor_copy(
                                kT_sb[:, j, 512 * i:512 * (i + 1)],
                                ps[:, 0:512],
                            )

                # ---- attention for pair j (interleaves with next pair) ----
                ci = ccinp.tile([128, T], F32R, tag="ci", name="ci")
                for qb in range(QBS):
                    qt = qtp.tile([128, 512], F32R, tag="qt")
                    nc.sync.dma_start(
                        out=qt[:], in_=qspill[j, :, 512 * qb:512 * (qb + 1)]
                    )
                    nchunks = 4 * (qb + 1)
                    augs = [
                        augAp.tile([128, 512], F32, tag="augA", name="augA"),
                        augBp.tile([128, 512], F32, tag="augB", name="augB"),
                    ]
                    for c in range(nchunks):
                        diag = c >= 4 * qb
                        o = (c - 4 * qb) * 128 if diag else 0
                        mo = min(o, 256)  # keep matmul free dim >= 256
                        last = c == nchunks - 1
                        s2 = pss.tile([128, 1024], F32, tag="s2", name="s2")
                        for hh in range(2):
                            nc.tensor.matmul(
                                s2[:, 512 * hh + mo:512 * hh + 512],
                                kT_sb[64 * hh:64 * hh + 64, j,
                                      128 * c:128 * (c + 1)],
                                qt[64 * hh:64 * hh + 64, mo:512],
                                start=True, stop=True,
                            )
                        aT = atp.tile([128, 1024], F32R, tag="aT")
                        nc.scalar.activation(
                            aT[:].rearrange("p (h q) -> p h q", h=2)[:, :, o:512],
                            s2[:].rearrange("p (h q) -> p h q", h=2)[:, :, o:512],
                            AF.Exp,
                        )
                        if diag:
                            for hh in range(2):
                                nc.vector.tensor_mul(
                                    aT[:, 512 * hh + o:512 * hh + o + 128],
                                    aT[:, 512 * hh + o:512 * hh + o + 128],
                                    mask_sb[:],
                                )
                        for hh in range(2):
                            nc.tensor.matmul(
                                augs[hh][0:HD + 1, o:512],
                                v_sb[:, c, 2 * j + hh, :],
                                aT[:, 512 * hh + o:512 * hh + 512],
                                start=(c == 0), stop=last,
                            )
                    nb = nbp.tile([128, 512], F32R, tag="nb")
                    for hh in range(2):
                        aug = augs[hh]
                        recip = normp.tile([128, 512], F32, tag="recip")
                        nc.vector.reciprocal(recip[0:1, :], aug[HD:HD + 1, :])
                        bc = normp.tile([64, 512], F32, tag="bc")
                        nc.gpsimd.partition_broadcast(
                            bc[:], recip[0:1, :], channels=64
                        )
                        nc.vector.tensor_mul(
                            nb[64 * hh:64 * (hh + 1), :], aug[0:HD, :], bc[:]
                        )
                    nc.sync.dma_start(
                        out=ci[:, 512 * qb:512 * (qb + 1)], in_=nb[:]
                    )
                co = ccoutp.tile([256, T], F32R, tag="co", name="co")
                if single_core:
                    # timing stand-in for the pairwise AllGather
                    nc.sync.dma_start(out=co[0:128, :], in_=ci[:])
                    nc.sync.dma_start(out=co[128:256, :], in_=ci[:])
                else:
                    nc.gpsimd.collective_compute(
                        "AllGather",
                        mybir.AluOpType.bypass,
                        replica_groups=REPLICA_GROUPS,
                        ins=[ci.opt()],
                        outs=[co.opt()],
                    )
                cc_outs.append(co)

        # ---------------- output projection ----------------
        with tc.tile_pool(name="apf", bufs=2 * NPAIR) as apf, \
             tc.tile_pool(name="ysb", bufs=3) as ysbp:
            att_tiles = []  # (c_in block index, tile)
            for j in range(NPAIR):
                for half in range(2):
                    t_ = apf.tile([128, T], F32R, tag="apf", name="apf")
                    for pc in range(2):
                        nc.sync.dma_start(
                            out=t_[:, 1024 * pc:1024 * (pc + 1)],
                            in_=cc_outs[j][128 * half:128 * (half + 1),
                                           1024 * pc:1024 * (pc + 1)],
                        )
                    att_tiles.append((4 * half + j, t_))
            for tb in range(TB):
                ps = pss.tile([128, 1024], F32, tag="s2", name="psy")
                for idx, (a, t_) in enumerate(att_tiles):
                    nc.tensor.matmul(
                        ps[:, 0:512],
                        t_[:, 128 * tb:128 * (tb + 1)],
                        wp_sb[:, a, :],
                        start=(idx == 0), stop=(idx == len(att_tiles) - 1),
                    )
                ysb = ysbp.tile([128, 512], F32, tag="ysb")
                nc.vector.tensor_copy(ysb[:], ps[:, 0:512])
                nc.sync.dma_start(
                    out=y[128 * tb:128 * (tb + 1), :], in_=ysb[:]
                )



_NC_CACHE = None


def _get_nc():
    global _NC_CACHE
    if _NC_CACHE is None:
        _NC_CACHE = build_nc()
    return _NC_CACHE


def _mask_np():
    # mask[kv', q'] = 1 where q' >= kv' (within-chunk causal triangle)
    kv = np.arange(128)[:, None]
    q = np.arange(128)[None, :]
    return (q >= kv).astype(np.float32)


def shard_inputs(x, w_qkv, w_proj):
    x = np.asarray(x, dtype=np.float32)
    w_qkv = np.asarray(w_qkv, dtype=np.float32)
    w_proj = np.asarray(w_proj, dtype=np.float32)
    mask = _mask_np()
    in_maps = []
    for core in range(N_CORES):
        pair, rank = divmod(core, 2)
        c0 = HD * HPC * rank  # 0 or 512: this core's head-column offset
        in_maps.append({
            "xt": np.ascontiguousarray(x[pair].T),
            "wq": np.ascontiguousarray(w_qkv[:, c0:c0 + 512]),
            "wk": np.ascontiguousarray(w_qkv[:, C + c0:C + c0 + 512]),
            "wv": np.ascontiguousarray(w_qkv[:, 2 * C + c0:2 * C + c0 + 512]),
            "wp": np.ascontiguousarray(w_proj[:, 512 * rank:512 * rank + 512]),
            "mask": mask,
        })
    return in_maps


def assemble_output(results):
    out = np.empty((B, T, C), dtype=np.float32)
    for core in range(N_CORES):
        pair, rank = divmod(core, 2)
        out[pair][:, 512 * rank:512 * rank + 512] = results[core]["y"]
    return out


# --- cached PJRT runner (same path run_bass_kernel_spmd takes under axon,
# but keeps the jitted executable so repeat calls skip re-tracing) ---
_RUNNER_CACHE = None


def _make_runner(nc):
    import jax
    import numpy as _np
    from jax.sharding import Mesh, PartitionSpec
    from jax.experimental.shard_map import shard_map
    from concourse import bass2jax
    from concourse.bass2jax import _bass_exec_p, install_neuronx_cc_hook

    install_neuronx_cc_hook()
    part_name = (nc.partition_id_tensor.name
                 if nc.partition_id_tensor else None)
    in_names, out_names, out_avals, zero_shapes = [], [], [], []
    for alloc in nc.m.functions[0].allocations:
        if not isinstance(alloc, mybir.MemoryLocationSet):
            continue
        name = alloc.memorylocations[0].name
        if alloc.kind == "ExternalInput":
            if name != part_name:
                in_names.append(name)
        elif alloc.kind == "ExternalOutput":
            out_names.append(name)
            shape = tuple(alloc.tensor_shape)
            dtype = mybir.dt.np(alloc.dtype)
            out_avals.append(jax.core.ShapedArray(shape, dtype))
            zero_shapes.append((shape, dtype))
    n_params = len(in_names)
    n_outs = len(out_names)
    all_in_names = in_names + out_names
    if part_name is not None:
        all_in_names = all_in_names + [part_name]

    def _body(*args):
        operands = list(args)
        if part_name is not None:
            operands.append(bass2jax.partition_id_tensor())
        outs = _bass_exec_p.bind(
            *operands,
            out_avals=tuple(out_avals),
            in_names=tuple(all_in_names),
            out_names=tuple(out_names),
            lowering_input_output_aliases=(),
            sim_require_finite=True,
            sim_require_nnan=True,
            nc=nc,
        )
        return tuple(outs)

    devices = jax.devices()[:N_CORES]
    mesh = Mesh(_np.asarray(devices), ("core",))
    in_specs = (PartitionSpec("core"),) * (n_params + n_outs)
    out_specs = (PartitionSpec("core"),) * n_outs
    donate = tuple(range(n_params, n_params + n_outs))
    sharded = jax.jit(
        shard_map(_body, mesh=mesh, in_specs=in_specs, out_specs=out_specs,
                  check_rep=False),
        donate_argnums=donate, keep_unused=True,
    )

    def run(in_maps):
        concat_in = [
            _np.concatenate([_np.asarray(in_maps[c][nm]) for c in
                             range(N_CORES)], axis=0)
            for nm in in_names
        ]
        concat_zeros = [
            _np.zeros((N_CORES * s[0], *s[1:]), d) for s, d in zero_shapes
        ]
        out_arrs = sharded(*concat_in, *concat_zeros)
        return [
            {nm: _np.asarray(out_arrs[i]).reshape(
                N_CORES, *out_avals[i].shape)[c]
             for i, nm in enumerate(out_names)}
            for c in range(N_CORES)
        ]

    run.sharded = sharded
    run.in_names = in_names
    run.zero_shapes = zero_shapes
    run.mesh = mesh
    return run


def _get_runner():
    global _RUNNER_CACHE
    if _RUNNER_CACHE is None:
        _RUNNER_CACHE = _make_runner(_get_nc())
    return _RUNNER_CACHE


def kernel(x, w_qkv, w_proj):
    in_maps = shard_inputs(x, w_qkv, w_proj)
    try:
        results = _get_runner()(in_maps)
    except Exception:
        res = run_bass_kernel_spmd(_get_nc(), in_maps, list(range(N_CORES)))
        results = res.results
    return assemble_output(results)



# revision 15
# speedup vs baseline: 1.0377x; 1.0377x over previous
"""Causal self-attention (B=4, T=2048, C=1024, NH=16) on 8 TRN2 NeuronCores.

Sharding (tensor-parallel heads x data-parallel batch):
  - 4 core-pairs: pair p = cores (2p, 2p+1) handles batch b = p.
  - Within a pair, rank 0 computes heads 0-7, rank 1 heads 8-15
    (w_qkv output columns split by head group).
  - After attention each core holds its half of attnT [512, T] (d-major).
    Pairwise AllGathers (one per (head-pair, q-block)) exchange the halves;
    each core computes a 512-column half of the output projection
    (w_proj column split), so no all-reduce is needed.  w_proj rows are
    host-permuted to [own-half | peer-half] so the device program is
    rank-independent.  Host concatenates the column halves.

Device algorithm (per core):
  Phase 1  q/k: fp8e4 DoubleRow matmuls (w_qkv columns scaled x16 on the
           host so fp8 sees ~N(0,0.5); the 1/(8*16*16) un-scale is folded
           into the exp).  Contraction 1024 = 4 passes of K=256
           ([128 part, 2, *] slot-major interleave).  Outputs kept
           SBUF-resident in bf16 (no DRAM spill).
           v: bf16 matmuls (fp8 on the value path costs ~2e-2 rel err,
           too close to the gate), t-major with a fused ones-column.
  Phase 2  Attention per (head-pair j, 512-wide q block qb):
           sT[kv,q] = kT.T @ q on the PE (two heads packed in row groups),
           exp on ScalarE straight out of PSUM into bf16 aT tiles,
           one static 128x128 triangle multiply per diagonal chunk,
           aug[65,q] += v_aug.T @ aT accumulated over kv chunks in PSUM.
           Row 64 of aug is the softmax denominator; normalize via DVE
           reciprocal + GpSimd partition-broadcast + DVE multiply, writing
           bf16 directly into the resident attn tile.
  Phase 3  Per-(j,qb) pairwise AllGather of [128,512] attn blocks (so the
           projection can start on a q-block as soon as all head-pairs
           finish it, instead of waiting for the whole attention), then
           y[t,512] += attnT.T @ w_proj accumulated over 8 c_in blocks.

All tensors bf16 except: fp8e4 for the q/k weight/activation inputs,
fp32 PSUM accumulation everywhere, fp32 y output.
"""

import numpy as np
import ml_dtypes

import concourse.bass as bass
import concourse.mybir as mybir
import concourse.tile as tile
from concourse import bacc
from concourse.bass_utils import run_bass_kernel_spmd
from concourse.alu_op_type import AluOpType

B, T, C = 4, 2048, 1024
NH, HD = 16, 64
N_CORES = 8
HPC = NH // 2          # heads per core
NPAIR = HPC // 2       # head-pairs per core
TB = T // 128          # 128-row t blocks
QBS = T // 512         # 512-wide q blocks
KC = C // 128          # 128-deep contraction chunks (bf16 path)
KP = C // 256          # 256-deep DoubleRow passes (fp8 path)

# q/k production path: "qk8" = both fp8 DoubleRow, "q8" = q fp8 / k bf16,
# "bf16" = both bf16.  fp8 is ~4x cheaper on the PE for that phase; each
# fp8 operand adds ~6e-3..2e-2 of softmax-suppressed quantization noise.
QK_MODE = "bf16"

WS = 16.0              # host-side scale on w_q / w_k before fp8 quantization
QS = WS if QK_MODE in ("qk8", "q8") else 1.0
KS = WS if QK_MODE == "qk8" else 1.0
SCALE_EXP = float(1.0 / (np.sqrt(HD) * QS * KS))   # exp(s_psum * SCALE_EXP)

F32 = mybir.dt.float32
BF16 = mybir.dt.bfloat16
F8 = mybir.dt.float8e4
AF = mybir.ActivationFunctionType
DR = mybir.MatmulPerfMode.DoubleRow
REPLICA_GROUPS = [[0, 1], [2, 3], [4, 5], [6, 7]]


def build_nc(reps=1, single_core=False):
    nc = bacc.Bacc(
        "TRN2", target_bir_lowering=False, debug=False,
        num_devices=(1 if single_core else N_CORES),
    )

    need_f8 = QS == WS or KS == WS
    xt8 = (nc.dram_tensor("xt8", [C, T], F8, kind="ExternalInput")
           if need_f8 else None)
    xtb = nc.dram_tensor("xtb", [C, T], BF16, kind="ExternalInput")
    wq = nc.dram_tensor("wq", [C, 512], F8 if QS == WS else BF16,
                        kind="ExternalInput")
    wk = nc.dram_tensor("wk", [C, 512], F8 if KS == WS else BF16,
                        kind="ExternalInput")
    wvb = nc.dram_tensor("wvb", [C, 512], BF16, kind="ExternalInput")
    wpb = nc.dram_tensor("wpb", [C, 512], BF16, kind="ExternalInput")
    mask = nc.dram_tensor("mask", [128, 128], BF16, kind="ExternalInput")
    y = nc.dram_tensor("y", [T, 512], F32, kind="ExternalOutput")

    with tile.TileContext(nc) as tc:
        for _rep in range(reps):
            _emit_one(nc, tc, xt8, xtb, wq, wk, wvb, wpb, mask, y,
                      single_core)

    nc.compile()
    return nc


def _emit_one(nc, tc, xt8, xtb, wq, wk, wvb, wpb, mask, y, single_core):
    with tc.tile_pool(name="persist", bufs=1) as persist, \
         tc.tile_pool(name="ccin", bufs=2 * NPAIR, space="DRAM") as ccinp, \
         tc.tile_pool(name="ccout", bufs=2 * NPAIR, space="DRAM") as ccoutp, \
         tc.tile_pool(name="aT", bufs=3) as atp, \
         tc.tile_pool(name="norm", bufs=4) as normp, \
         tc.tile_pool(name="ysb", bufs=2) as ysbp, \
         tc.tile_pool(name="big", bufs=2, space="PSUM") as bigp, \
         tc.tile_pool(name="augA", bufs=2, space="PSUM") as augAp, \
         tc.tile_pool(name="augB", bufs=2, space="PSUM") as augBp:

        kT_sb = persist.tile([128, NPAIR, T], BF16)
        q_sb = persist.tile([128, NPAIR, T], BF16)
        v_sb = persist.tile([128, TB, HPC, HD + 1], BF16)
        attn_own = persist.tile([128, NPAIR, T], BF16)
        # both pair halves of attnT, in global rank order (rank-independent)
        apf_sb = persist.tile([128, 2 * NPAIR, T], BF16)
        wp_sb = persist.tile([128, KC, 512], BF16)
        mask_sb = persist.tile([128, 128], BF16)
        nc.sync.dma_start(out=mask_sb[:], in_=mask[:])
        nc.sync.dma_start(
            out=wp_sb[:], in_=wpb[:].rearrange("(a p) n -> p a n", p=128))

        ones_sc = normp.tile([128, TB * HPC], BF16, tag="ones")
        nc.vector.memset(ones_sc[:], 1.0)
        nc.vector.tensor_copy(
            v_sb[:, :, :, HD],
            ones_sc[:].rearrange("p (a b) -> p a b", a=TB),
        )

        # ---------------- phase 1: qkv ----------------
        with tc.tile_pool(name="xt8p", bufs=1) as xt8p, \
             tc.tile_pool(name="xtbp", bufs=1) as xtbp, \
             tc.tile_pool(name="wqkp", bufs=1) as wqkp, \
             tc.tile_pool(name="wvp", bufs=1) as wvp:
            # weight tiles: fp8 DoubleRow-sliced [p, pass, slot, *] or bf16
            def load_w(dram, is_f8, tag):
                if is_f8:
                    t = wqkp.tile([128, KP, 2, 512], F8, tag=tag)
                    nc.sync.dma_start(
                        out=t[:], in_=dram[:].rearrange(
                            "(a s p) n -> p a s n", p=128, s=2))
                else:
                    t = wqkp.tile([128, KC, 512], BF16, tag=tag)
                    nc.sync.dma_start(
                        out=t[:], in_=dram[:].rearrange(
                            "(a p) n -> p a n", p=128))
                return t

            wq_sb = load_w(wq, QS == WS, "wq")
            wk_sb = load_w(wk, KS == WS, "wk")
            if xt8 is not None:
                x8_sb = xt8p.tile([128, KP, 2, T], F8)
                x8_r = xt8[:].rearrange("(a s p) t -> p a s t", p=128, s=2)
                for a in range(KP):
                    nc.sync.dma_start(out=x8_sb[:, a], in_=x8_r[:, a])
            wv_sb = wvp.tile([128, KC, 512], BF16)
            nc.sync.dma_start(
                out=wv_sb[:], in_=wvb[:].rearrange("(a p) n -> p a n", p=128))
            xb_sb = xtbp.tile([128, KC, T], BF16)
            xb_r = xtb[:].rearrange("(a p) t -> p a t", p=128)
            for a in range(KC):
                nc.sync.dma_start(out=xb_sb[:, a], in_=xb_r[:, a])

            # q/k for all head-pairs
            for j in range(NPAIR):
                for wsb, is_f8, dst in ((wq_sb, QS == WS, q_sb),
                                        (wk_sb, KS == WS, kT_sb)):
                    for i in range(QBS):
                        ps = bigp.tile([128, 1024], F32, tag="big",
                                       name="psqk")
                        if is_f8:
                            for a in range(KP):
                                nc.tensor.matmul(
                                    ps[:, 0:512],
                                    wsb[:, a, :, 128 * j:128 * (j + 1)],
                                    x8_sb[:, a, :, 512 * i:512 * (i + 1)],
                                    start=(a == 0), stop=(a == KP - 1),
                                    perf_mode=DR,
                                )
                        else:
                            for a in range(KC):
                                nc.tensor.matmul(
                                    ps[:, 0:512],
                                    wsb[:, a, 128 * j:128 * (j + 1)],
                                    xb_sb[:, a, 512 * i:512 * (i + 1)],
                                    start=(a == 0), stop=(a == KC - 1),
                                )
                        nc.vector.tensor_copy(
                            dst[:, j, 512 * i:512 * (i + 1)], ps[:, 0:512])

            # v (bf16), by head-pair so attention j can start early
            for j in range(NPAIR):
                for tb in range(TB):
                    ps = bigp.tile([128, 1024], F32, tag="big", name="psv")
                    for a in range(KC):
                        nc.tensor.matmul(
                            ps[:, 0:128],
                            xb_sb[:, a, 128 * tb:128 * (tb + 1)],
                            wv_sb[:, a, 128 * j:128 * (j + 1)],
                            start=(a == 0), stop=(a == KC - 1),
                        )
                    nc.vector.tensor_copy(
                        v_sb[:, tb, 2 * j:2 * j + 2, 0:HD],
                        ps[:, 0:128].rearrange("p (h d) -> p h d", h=2),
                    )

        # ---------------- phase 2+3: attention + allgather ----------------
        # qb-outer so the projection for q-block qb can run as soon as all
        # head-pairs finish it; proj(qb) is emitted after attention(qb+1) so
        # it only fills PE bubbles instead of delaying the exp pipeline.
        def proj_block(qb):
            for tt in range(4):
                t0 = 512 * qb + 128 * tt
                ps = bigp.tile([128, 1024], F32, tag="big", name="psy")
                for idx in range(2 * NPAIR):
                    nc.tensor.matmul(
                        ps[:, 0:512],
                        apf_sb[:, idx, t0:t0 + 128],
                        wp_sb[:, idx, :],
                        start=(idx == 0), stop=(idx == 7),
                    )
                ysb = ysbp.tile([128, 512], F32, tag="ysb")
                nc.vector.tensor_copy(ysb[:], ps[:, 0:512])
                nc.sync.dma_start(out=y[t0:t0 + 128, :], in_=ysb[:])

        cc_outs = {}
        for qb in range(QBS):
            for j in range(NPAIR):
                nchunks = 4 * (qb + 1)
                augs = [
                    augAp.tile([HD + 1, 512], F32, tag="augA", name="augA"),
                    augBp.tile([HD + 1, 512], F32, tag="augB", name="augB"),
                ]
                for c in range(nchunks):
                    diag = c >= 4 * qb
                    o = (c - 4 * qb) * 128 if diag else 0
                    last = c == nchunks - 1
                    s2 = bigp.tile([128, 1024], F32, tag="big", name="s2")
                    for hh in range(2):
                        nc.tensor.matmul(
                            s2[:, 512 * hh + o:512 * hh + 512],
                            kT_sb[64 * hh:64 * hh + 64, j,
                                  128 * c:128 * (c + 1)],
                            q_sb[64 * hh:64 * hh + 64, j,
                                 512 * qb + o:512 * (qb + 1)],
                            start=True, stop=True,
                        )
                    aT = atp.tile([128, 2, 512], BF16, tag="aT")
                    nc.scalar.activation(
                        aT[:, :, o:512],
                        s2[:].rearrange("p (h q) -> p h q", h=2)[:, :, o:512],
                        AF.Exp, scale=SCALE_EXP,
                    )
                    if diag:
                        for hh in range(2):
                            eng = nc.vector if (c % 2 == 0) else nc.gpsimd
                            eng.tensor_mul(
                                aT[:, hh, o:o + 128],
                                aT[:, hh, o:o + 128],
                                mask_sb[:],
                            )
                    for hh in range(2):
                        nc.tensor.matmul(
                            augs[hh][0:HD + 1, o:512],
                            v_sb[:, c, 2 * j + hh, :],
                            aT[:, hh, o:512],
                            start=(c == 0), stop=last,
                        )
                # normalize into the resident attn tile
                for hh in range(2):
                    aug = augs[hh]
                    recip = normp.tile([1, 512], F32, tag="recip")
                    nc.vector.reciprocal(recip[:], aug[HD:HD + 1, :])
                    bc = normp.tile([64, 512], F32, tag="bc")
                    nc.gpsimd.partition_broadcast(bc[:], recip[:], channels=64)
                    nc.vector.tensor_mul(
                        attn_own[64 * hh:64 * (hh + 1), j,
                                 512 * qb:512 * (qb + 1)],
                        aug[0:HD, :], bc[:],
                    )
                # exchange this (j, qb) block with the pair peer
                ci = ccinp.tile([128, 512], BF16, tag="ci", name="ci")
                nc.sync.dma_start(
                    out=ci[:],
                    in_=attn_own[:, j, 512 * qb:512 * (qb + 1)],
                )
                co = ccoutp.tile([256, 512], BF16, tag="co", name="co")
                if single_core:
                    # timing stand-in for the pairwise AllGather
                    nc.sync.dma_start(out=co[0:128, :], in_=ci[:])
                    nc.sync.dma_start(out=co[128:256, :], in_=ci[:])
                else:
                    nc.gpsimd.collective_compute(
                        "AllGather",
                        mybir.AluOpType.bypass,
                        replica_groups=REPLICA_GROUPS,
                        ins=[ci.opt()],
                        outs=[co.opt()],
                    )
                cc_outs[(j, qb)] = co
                # both halves of this (j, qb) attn block, rank-major
                for half in range(2):
                    nc.sync.dma_start(
                        out=apf_sb[:, NPAIR * half + j,
                                   512 * qb:512 * (qb + 1)],
                        in_=co[128 * half:128 * (half + 1), :],
                    )
            if qb >= 1:
                proj_block(qb - 1)
        proj_block(QBS - 1)


_NC_CACHE = None


def _get_nc():
    global _NC_CACHE
    if _NC_CACHE is None:
        _NC_CACHE = build_nc()
    return _NC_CACHE


def _mask_np():
    # mask[kv', q'] = 1 where q' >= kv' (within-chunk causal triangle)
    kv = np.arange(128)[:, None]
    q = np.arange(128)[None, :]
    return (q >= kv).astype(ml_dtypes.bfloat16)


def shard_inputs(x, w_qkv, w_proj):
    x = np.asarray(x, dtype=np.float32)
    w_qkv = np.asarray(w_qkv, dtype=np.float32)
    w_proj = np.asarray(w_proj, dtype=np.float32)
    mask = _mask_np()
    in_maps = []
    for core in range(N_CORES):
        pair, rank = divmod(core, 2)
        c0 = HD * HPC * rank  # 0 or 512: this core's head-column offset
        xt = np.ascontiguousarray(x[pair].T)
        wp = w_proj[:, 512 * rank:512 * rank + 512]

        def wslice(idx, scale, f8):
            w = w_qkv[:, idx * C + c0:idx * C + c0 + 512] * scale
            return w.astype(ml_dtypes.float8_e4m3 if f8 else ml_dtypes.bfloat16)

        m = {
            "xtb": xt.astype(ml_dtypes.bfloat16),
            "wq": wslice(0, QS, QS == WS),
            "wk": wslice(1, KS, KS == WS),
            "wvb": wslice(2, 1.0, False),
            "wpb": np.ascontiguousarray(wp).astype(ml_dtypes.bfloat16),
            "mask": mask,
        }
        if QS == WS or KS == WS:
            m["xt8"] = xt.astype(ml_dtypes.float8_e4m3)
        in_maps.append(m)
    return in_maps


def assemble_output(results):
    out = np.empty((B, T, C), dtype=np.float32)
    for core in range(N_CORES):
        pair, rank = divmod(core, 2)
        out[pair][:, 512 * rank:512 * rank + 512] = results[core]["y"]
    return out


# --- cached PJRT runner (same path run_bass_kernel_spmd takes under axon,
# but keeps the jitted executable so repeat calls skip re-tracing) ---
_RUNNER_CACHE = None


def _make_runner(nc):
    import jax
    import numpy as _np
    from jax.sharding import Mesh, PartitionSpec
    from jax.experimental.shard_map import shard_map
    from concourse import bass2jax
    from concourse.bass2jax import _bass_exec_p, install_neuronx_cc_hook

    install_neuronx_cc_hook()
    part_name = (nc.partition_id_tensor.name
                 if nc.partition_id_tensor else None)
    in_names, out_names, out_avals, zero_shapes = [], [], [], []
    for alloc in nc.m.functions[0].allocations:
        if not isinstance(alloc, mybir.MemoryLocationSet):
            continue
        name = alloc.memorylocations[0].name
        if alloc.kind == "ExternalInput":
            if name != part_name:
                in_names.append(name)
        elif alloc.kind == "ExternalOutput":
            out_names.append(name)
            shape = tuple(alloc.tensor_shape)
            dtype = mybir.dt.np(alloc.dtype)
            out_avals.append(jax.core.ShapedArray(shape, dtype))
            zero_shapes.append((shape, dtype))
    n_params = len(in_names)
    n_outs = len(out_names)
    all_in_names = in_names + out_names
    if part_name is not None:
        all_in_names = all_in_names + [part_name]

    def _body(*args):
        operands = list(args)
        if part_name is not None:
            operands.append(bass2jax.partition_id_tensor())
        outs = _bass_exec_p.bind(
            *operands,
            out_avals=tuple(out_avals),
            in_names=tuple(all_in_names),
            out_names=tuple(out_names),
            lowering_input_output_aliases=(),
            sim_require_finite=True,
            sim_require_nnan=True,
            nc=nc,
        )
        return tuple(outs)

    devices = jax.devices()[:N_CORES]
    mesh = Mesh(_np.asarray(devices), ("core",))
    in_specs = (PartitionSpec("core"),) * (n_params + n_outs)
    out_specs = (PartitionSpec("core"),) * n_outs
    donate = tuple(range(n_params, n_params + n_outs))
    sharded = jax.jit(
        shard_map(_body, mesh=mesh, in_specs=in_specs, out_specs=out_specs,
                  check_rep=False),
        donate_argnums=donate, keep_unused=True,
    )

    def run(in_maps):
        concat_in = [
            _np.concatenate([_np.asarray(in_maps[c][nm]) for c in
                             range(N_CORES)], axis=0)
            for nm in in_names
        ]
        concat_zeros = [
            _np.zeros((N_CORES * s[0], *s[1:]), d) for s, d in zero_shapes
        ]
        out_arrs = sharded(*concat_in, *concat_zeros)
        return [
            {nm: _np.asarray(out_arrs[i]).reshape(
                N_CORES, *out_avals[i].shape)[c]
             for i, nm in enumerate(out_names)}
            for c in range(N_CORES)
        ]

    run.sharded = sharded
    run.in_names = in_names
    run.zero_shapes = zero_shapes
    run.mesh = mesh
    return run


def _get_runner():
    global _RUNNER_CACHE
    if _RUNNER_CACHE is None:
        _RUNNER_CACHE = _make_runner(_get_nc())
    return _RUNNER_CACHE


def kernel(x, w_qkv, w_proj):
    in_maps = shard_inputs(x, w_qkv, w_proj)
    try:
        results = _get_runner()(in_maps)
    except Exception:
        res = run_bass_kernel_spmd(_get_nc(), in_maps, list(range(N_CORES)))
        results = res.results
    return assemble_output(results)


# revision 19
# speedup vs baseline: 1.0931x; 1.0534x over previous
"""Causal self-attention (B=4, T=2048, C=1024, NH=16) on 8 TRN2 NeuronCores.

Sharding (tensor-parallel heads x data-parallel batch):
  - 4 core-pairs: pair p = cores (2p, 2p+1) handles batch b = p.
  - Within a pair, rank 0 computes heads 0-7, rank 1 heads 8-15
    (w_qkv output columns split by head group).
  - After attention each core holds its half of attnT [512, T] (d-major).
    Pairwise AllGathers (one per (head-pair, q-block)) exchange the halves;
    each core computes a 512-column half of the output projection
    (w_proj column split), so no all-reduce is needed.  w_proj rows are
    host-permuted to [own-half | peer-half] so the device program is
    rank-independent.  Host concatenates the column halves.

Device algorithm (per core):
  Phase 1  q/k: fp8e4 DoubleRow matmuls (w_qkv columns scaled x16 on the
           host so fp8 sees ~N(0,0.5); the 1/(8*16*16) un-scale is folded
           into the exp).  Contraction 1024 = 4 passes of K=256
           ([128 part, 2, *] slot-major interleave).  Outputs kept
           SBUF-resident in bf16 (no DRAM spill).
           v: bf16 matmuls (fp8 on the value path costs ~2e-2 rel err,
           too close to the gate), t-major with a fused ones-column.
  Phase 2  Attention per (head-pair j, 512-wide q block qb):
           sT[kv,q] = kT.T @ q on the PE (two heads packed in row groups),
           exp on ScalarE straight out of PSUM into bf16 aT tiles,
           one static 128x128 triangle multiply per diagonal chunk,
           aug[65,q] += v_aug.T @ aT accumulated over kv chunks in PSUM.
           Row 64 of aug is the softmax denominator; normalize via DVE
           reciprocal + GpSimd partition-broadcast + DVE multiply, writing
           bf16 directly into the resident attn tile.
  Phase 3  Per-(j,qb) pairwise AllGather of [128,512] attn blocks (so the
           projection can start on a q-block as soon as all head-pairs
           finish it, instead of waiting for the whole attention), then
           y[t,512] += attnT.T @ w_proj accumulated over 8 c_in blocks.

All tensors bf16 except: fp8e4 for the q/k weight/activation inputs,
fp32 PSUM accumulation everywhere, fp32 y output.
"""

import numpy as np
import ml_dtypes

import concourse.bass as bass
import concourse.mybir as mybir
import concourse.tile as tile
from concourse import bacc
from concourse.bass_utils import run_bass_kernel_spmd
from concourse.alu_op_type import AluOpType

B, T, C = 4, 2048, 1024
NH, HD = 16, 64
N_CORES = 8
HPC = NH // 2          # heads per core
NPAIR = HPC // 2       # head-pairs per core
TB = T // 128          # 128-row t blocks
QBS = T // 512         # 512-wide q blocks
KC = C // 128          # 128-deep contraction chunks (bf16 path)
KP = C // 256          # 256-deep DoubleRow passes (fp8 path)

# q/k production path: "qk8" = both fp8 DoubleRow, "q8" = q fp8 / k bf16,
# "bf16" = both bf16.  fp8 is ~4x cheaper on the PE for that phase; each
# fp8 operand adds ~6e-3..2e-2 of softmax-suppressed quantization noise.
QK_MODE = "bf16"

WS = 16.0              # host-side scale on w_q / w_k before fp8 quantization
QS = WS if QK_MODE in ("qk8", "q8") else 1.0
KS = WS if QK_MODE == "qk8" else 1.0
SCALE_EXP = float(1.0 / (np.sqrt(HD) * QS * KS))   # exp(s_psum * SCALE_EXP)

F32 = mybir.dt.float32
BF16 = mybir.dt.bfloat16
F8 = mybir.dt.float8e4
AF = mybir.ActivationFunctionType
DR = mybir.MatmulPerfMode.DoubleRow
REPLICA_GROUPS = [[0, 1], [2, 3], [4, 5], [6, 7]]


def build_nc(reps=1, single_core=False):
    nc = bacc.Bacc(
        "TRN2", target_bir_lowering=False, debug=False,
        num_devices=(1 if single_core else N_CORES),
    )

    need_f8 = QS == WS or KS == WS
    xt8 = (nc.dram_tensor("xt8", [C, T], F8, kind="ExternalInput")
           if need_f8 else None)
    xtb = nc.dram_tensor("xtb", [C, T], BF16, kind="ExternalInput")
    wq = nc.dram_tensor("wq", [C, 512], F8 if QS == WS else BF16,
                        kind="ExternalInput")
    wk = nc.dram_tensor("wk", [C, 512], F8 if KS == WS else BF16,
                        kind="ExternalInput")
    wvb = nc.dram_tensor("wvb", [C, 512], BF16, kind="ExternalInput")
    wpb = nc.dram_tensor("wpb", [C, 512], BF16, kind="ExternalInput")
    mask = nc.dram_tensor("mask", [128, 128], BF16, kind="ExternalInput")
    y = nc.dram_tensor("y", [T, 512], F32, kind="ExternalOutput")

    with tile.TileContext(nc) as tc:
        for _rep in range(reps):
            _emit_one(nc, tc, xt8, xtb, wq, wk, wvb, wpb, mask, y,
                      single_core)

    nc.compile()
    return nc


def _emit_one(nc, tc, xt8, xtb, wq, wk, wvb, wpb, mask, y, single_core):
    with tc.tile_pool(name="persist", bufs=1) as persist, \
         tc.tile_pool(name="ccin", bufs=2 * NPAIR, space="DRAM") as ccinp, \
         tc.tile_pool(name="ccout", bufs=2 * NPAIR, space="DRAM") as ccoutp, \
         tc.tile_pool(name="aT", bufs=4) as atp, \
         tc.tile_pool(name="norm", bufs=4) as normp, \
         tc.tile_pool(name="augc", bufs=4) as augcp, \
         tc.tile_pool(name="ysb", bufs=2) as ysbp, \
         tc.tile_pool(name="big", bufs=3, space="PSUM") as bigp, \
         tc.tile_pool(name="augA", bufs=1, space="PSUM") as augAp, \
         tc.tile_pool(name="augB", bufs=1, space="PSUM") as augBp:

        kT_sb = persist.tile([128, NPAIR, T], BF16)
        q_sb = persist.tile([128, NPAIR, T], BF16)
        v_sb = persist.tile([128, TB, HPC, HD + 1], BF16)
        attn_own = persist.tile([128, NPAIR, T], BF16)
        # both pair halves of attnT, in global rank order (rank-independent)
        apf_sb = persist.tile([128, 2 * NPAIR, T], BF16)
        wp_sb = persist.tile([128, KC, 512], BF16)
        mask_sb = persist.tile([128, 128], BF16)

        ones_sc = normp.tile([128, TB * HPC], BF16, tag="ones")
        nc.vector.memset(ones_sc[:], 1.0)
        nc.vector.tensor_copy(
            v_sb[:, :, :, HD],
            ones_sc[:].rearrange("p (a b) -> p a b", a=TB),
        )

        # ---------------- phase 1: qkv ----------------
        with tc.tile_pool(name="xt8p", bufs=1) as xt8p, \
             tc.tile_pool(name="xtbp", bufs=1) as xtbp, \
             tc.tile_pool(name="wqkp", bufs=1) as wqkp, \
             tc.tile_pool(name="wvp", bufs=1) as wvp:
            # weight tiles: fp8 DoubleRow-sliced [p, pass, slot, *] or bf16
            def load_w(dram, is_f8, tag):
                if is_f8:
                    t = wqkp.tile([128, KP, 2, 512], F8, tag=tag)
                    nc.sync.dma_start(
                        out=t[:], in_=dram[:].rearrange(
                            "(a s p) n -> p a s n", p=128, s=2))
                else:
                    t = wqkp.tile([128, KC, 512], BF16, tag=tag)
                    nc.sync.dma_start(
                        out=t[:], in_=dram[:].rearrange(
                            "(a p) n -> p a n", p=128))
                return t

            wq_sb = load_w(wq, QS == WS, "wq")
            wk_sb = load_w(wk, KS == WS, "wk")
            wv_sb = wvp.tile([128, KC, 512], BF16)
            nc.sync.dma_start(
                out=wv_sb[:], in_=wvb[:].rearrange("(a p) n -> p a n", p=128))
            if xt8 is not None:
                x8_sb = xt8p.tile([128, KP, 2, T], F8)
                x8_r = xt8[:].rearrange("(a s p) t -> p a s t", p=128, s=2)
                for a in range(KP):
                    nc.sync.dma_start(out=x8_sb[:, a], in_=x8_r[:, a])
            # x by (t-block-of-512, chunk) so early q/k/v tiles unblock after
            # the first quarter of the transfer
            xb_sb = xtbp.tile([128, KC, T], BF16)
            xb_r = xtb[:].rearrange("(a p) t -> p a t", p=128)
            for i in range(QBS):
                for a in range(KC):
                    nc.sync.dma_start(
                        out=xb_sb[:, a, 512 * i:512 * (i + 1)],
                        in_=xb_r[:, a, 512 * i:512 * (i + 1)])
            # low-priority loads last
            nc.sync.dma_start(out=mask_sb[:], in_=mask[:])
            nc.sync.dma_start(
                out=wp_sb[:], in_=wpb[:].rearrange("(a p) n -> p a n", p=128))

            # q/k/v per head-pair so attention on pair j can start early
            for j in range(NPAIR):
                for wsb, is_f8, dst in ((wq_sb, QS == WS, q_sb),
                                        (wk_sb, KS == WS, kT_sb)):
                    for i in range(QBS):
                        ps = bigp.tile([128, 1024], F32, tag="big",
                                       name="psqk")
                        if is_f8:
                            for a in range(KP):
                                nc.tensor.matmul(
                                    ps[:, 0:512],
                                    wsb[:, a, :, 128 * j:128 * (j + 1)],
                                    x8_sb[:, a, :, 512 * i:512 * (i + 1)],
                                    start=(a == 0), stop=(a == KP - 1),
                                    perf_mode=DR,
                                )
                        else:
                            for a in range(KC):
                                nc.tensor.matmul(
                                    ps[:, 0:512],
                                    wsb[:, a, 128 * j:128 * (j + 1)],
                                    xb_sb[:, a, 512 * i:512 * (i + 1)],
                                    start=(a == 0), stop=(a == KC - 1),
                                )
                        nc.vector.tensor_copy(
                            dst[:, j, 512 * i:512 * (i + 1)], ps[:, 0:512])
                for tb in range(TB):
                    ps = bigp.tile([128, 1024], F32, tag="big", name="psv")
                    for a in range(KC):
                        nc.tensor.matmul(
                            ps[:, 0:128],
                            xb_sb[:, a, 128 * tb:128 * (tb + 1)],
                            wv_sb[:, a, 128 * j:128 * (j + 1)],
                            start=(a == 0), stop=(a == KC - 1),
                        )
                    nc.vector.tensor_copy(
                        v_sb[:, tb, 2 * j:2 * j + 2, 0:HD],
                        ps[:, 0:128].rearrange("p (h d) -> p h d", h=2),
                    )

        # ---------------- phase 2+3: attention + allgather ----------------
        # qb-outer so the projection for q-block qb can run as soon as all
        # head-pairs finish it; proj(qb) is emitted after attention(qb+1) so
        # it only fills PE bubbles instead of delaying the exp pipeline.
        def proj_block(qb):
            for tt in range(4):
                t0 = 512 * qb + 128 * tt
                ps = bigp.tile([128, 1024], F32, tag="big", name="psy")
                for idx in range(2 * NPAIR):
                    nc.tensor.matmul(
                        ps[:, 0:512],
                        apf_sb[:, idx, t0:t0 + 128],
                        wp_sb[:, idx, :],
                        start=(idx == 0), stop=(idx == 7),
                    )
                ysb = ysbp.tile([128, 512], F32, tag="ysb")
                nc.vector.tensor_copy(ysb[:], ps[:, 0:512])
                nc.sync.dma_start(out=y[t0:t0 + 128, :], in_=ysb[:])

        cc_outs = {}
        for qb in range(QBS):
            for j in range(NPAIR):
                nchunks = 4 * (qb + 1)
                augs = [
                    augAp.tile([HD + 1, 512], F32, tag="augA", name="augA"),
                    augBp.tile([HD + 1, 512], F32, tag="augB", name="augB"),
                ]
                for c in range(nchunks):
                    diag = c >= 4 * qb
                    o = (c - 4 * qb) * 128 if diag else 0
                    last = c == nchunks - 1
                    s2 = bigp.tile([128, 1024], F32, tag="big", name="s2")
                    for hh in range(2):
                        nc.tensor.matmul(
                            s2[:, 512 * hh + o:512 * hh + 512],
                            kT_sb[64 * hh:64 * hh + 64, j,
                                  128 * c:128 * (c + 1)],
                            q_sb[64 * hh:64 * hh + 64, j,
                                 512 * qb + o:512 * (qb + 1)],
                            start=True, stop=True,
                        )
                    aT = atp.tile([128, 2, 512], BF16, tag="aT")
                    nc.scalar.activation(
                        aT[:, :, o:512],
                        s2[:].rearrange("p (h q) -> p h q", h=2)[:, :, o:512],
                        AF.Exp, scale=SCALE_EXP,
                    )
                    if diag:
                        for hh in range(2):
                            eng = nc.vector if (c % 2 == 0) else nc.gpsimd
                            eng.tensor_mul(
                                aT[:, hh, o:o + 128],
                                aT[:, hh, o:o + 128],
                                mask_sb[:],
                            )
                    for hh in range(2):
                        nc.tensor.matmul(
                            augs[hh][0:HD + 1, o:512],
                            v_sb[:, c, 2 * j + hh, :],
                            aT[:, hh, o:512],
                            start=(c == 0), stop=last,
                        )
                # normalize into the resident attn tile.  aug is copied out
                # of PSUM right away so the (bufs=1) aug banks free up for
                # the next head-pair while the normalize chain runs.
                for hh in range(2):
                    aug = augs[hh]
                    augc = augcp.tile([HD + 1, 512], F32, tag="augc")
                    nc.vector.tensor_copy(augc[:], aug[:])
                    recip = normp.tile([1, 512], F32, tag="recip")
                    nc.vector.reciprocal(recip[:], augc[HD:HD + 1, :])
                    bc = normp.tile([64, 512], F32, tag="bc")
                    nc.gpsimd.partition_broadcast(bc[:], recip[:], channels=64)
                    nc.vector.tensor_mul(
                        attn_own[64 * hh:64 * (hh + 1), j,
                                 512 * qb:512 * (qb + 1)],
                        augc[0:HD, :], bc[:],
                    )
                # exchange this (j, qb) block with the pair peer
                ci = ccinp.tile([128, 512], BF16, tag="ci", name="ci")
                nc.sync.dma_start(
                    out=ci[:],
                    in_=attn_own[:, j, 512 * qb:512 * (qb + 1)],
                )
                co = ccoutp.tile([256, 512], BF16, tag="co", name="co")
                if single_core:
                    # timing stand-in for the pairwise AllGather
                    nc.sync.dma_start(out=co[0:128, :], in_=ci[:])
                    nc.sync.dma_start(out=co[128:256, :], in_=ci[:])
                else:
                    nc.gpsimd.collective_compute(
                        "AllGather",
                        mybir.AluOpType.bypass,
                        replica_groups=REPLICA_GROUPS,
                        ins=[ci.opt()],
                        outs=[co.opt()],
                    )
                cc_outs[(j, qb)] = co
                # both halves of this (j, qb) attn block, rank-major
                for half in range(2):
                    nc.sync.dma_start(
                        out=apf_sb[:, NPAIR * half + j,
                                   512 * qb:512 * (qb + 1)],
                        in_=co[128 * half:128 * (half + 1), :],
                    )
            if qb >= 1:
                proj_block(qb - 1)
        proj_block(QBS - 1)


_NC_CACHE = None


def _get_nc():
    global _NC_CACHE
    if _NC_CACHE is None:
        _NC_CACHE = build_nc()
    return _NC_CACHE


def _mask_np():
    # mask[kv', q'] = 1 where q' >= kv' (within-chunk causal triangle)
    kv = np.arange(128)[:, None]
    q = np.arange(128)[None, :]
    return (q >= kv).astype(ml_dtypes.bfloat16)


def shard_inputs(x, w_qkv, w_proj):
    x = np.asarray(x, dtype=np.float32)
    w_qkv = np.asarray(w_qkv, dtype=np.float32)
    w_proj = np.asarray(w_proj, dtype=np.float32)
    mask = _mask_np()
    in_maps = []
    for core in range(N_CORES):
        pair, rank = divmod(core, 2)
        c0 = HD * HPC * rank  # 0 or 512: this core's head-column offset
        xt = np.ascontiguousarray(x[pair].T)
        wp = w_proj[:, 512 * rank:512 * rank + 512]

        def wslice(idx, scale, f8):
            w = w_qkv[:, idx * C + c0:idx * C + c0 + 512] * scale
            return w.astype(ml_dtypes.float8_e4m3 if f8 else ml_dtypes.bfloat16)

        m = {
            "xtb": xt.astype(ml_dtypes.bfloat16),
            "wq": wslice(0, QS, QS == WS),
            "wk": wslice(1, KS, KS == WS),
            "wvb": wslice(2, 1.0, False),
            "wpb": np.ascontiguousarray(wp).astype(ml_dtypes.bfloat16),
            "mask": mask,
        }
        if QS == WS or KS == WS:
            m["xt8"] = xt.astype(ml_dtypes.float8_e4m3)
        in_maps.append(m)
    return in_maps


def assemble_output(results):
    out = np.empty((B, T, C), dtype=np.float32)
    for core in range(N_CORES):
        pair, rank = divmod(core, 2)
        out[pair][:, 512 * rank:512 * rank + 512] = results[core]["y"]
    return out


# --- cached PJRT runner (same path run_bass_kernel_spmd takes under axon,
# but keeps the jitted executable so repeat calls skip re-tracing) ---
_RUNNER_CACHE = None


def _make_runner(nc):
    import jax
    import numpy as _np
    from jax.sharding import Mesh, PartitionSpec
    from jax.experimental.shard_map import shard_map
    from concourse import bass2jax
    from concourse.bass2jax import _bass_exec_p, install_neuronx_cc_hook

    install_neuronx_cc_hook()
    part_name = (nc.partition_id_tensor.name
                 if nc.partition_id_tensor else None)
    in_names, out_names, out_avals, zero_shapes = [], [], [], []
    for alloc in nc.m.functions[0].allocations:
        if not isinstance(alloc, mybir.MemoryLocationSet):
            continue
        name = alloc.memorylocations[0].name
        if alloc.kind == "ExternalInput":
            if name != part_name:
                in_names.append(name)
        elif alloc.kind == "ExternalOutput":
            out_names.append(name)
            shape = tuple(alloc.tensor_shape)
            dtype = mybir.dt.np(alloc.dtype)
            out_avals.append(jax.core.ShapedArray(shape, dtype))
            zero_shapes.append((shape, dtype))
    n_params = len(in_names)
    n_outs = len(out_names)
    all_in_names = in_names + out_names
    if part_name is not None:
        all_in_names = all_in_names + [part_name]

    def _body(*args):
        operands = list(args)
        if part_name is not None:
            operands.append(bass2jax.partition_id_tensor())
        outs = _bass_exec_p.bind(
            *operands,
            out_avals=tuple(out_avals),
            in_names=tuple(all_in_names),
            out_names=tuple(out_names),
            lowering_input_output_aliases=(),
            sim_require_finite=True,
            sim_require_nnan=True,
            nc=nc,
        )
        return tuple(outs)

    devices = jax.devices()[:N_CORES]
    mesh = Mesh(_np.asarray(devices), ("core",))
    in_specs = (PartitionSpec("core"),) * (n_params + n_outs)
    out_specs = (PartitionSpec("core"),) * n_outs
    donate = tuple(range(n_params, n_params + n_outs))
    sharded = jax.jit(
        shard_map(_body, mesh=mesh, in_specs=in_specs, out_specs=out_specs,
                  check_rep=False),
        donate_argnums=donate, keep_unused=True,
    )

    def run(in_maps):
        concat_in = [
            _np.concatenate([_np.asarray(in_maps[c][nm]) for c in
                             range(N_CORES)], axis=0)
            for nm in in_names
        ]
        concat_zeros = [
            _np.zeros((N_CORES * s[0], *s[1:]), d) for s, d in zero_shapes
        ]
        out_arrs = sharded(*concat_in, *concat_zeros)
        return [
            {nm: _np.asarray(out_arrs[i]).reshape(
                N_CORES, *out_avals[i].shape)[c]
             for i, nm in enumerate(out_names)}
            for c in range(N_CORES)
        ]

    run.sharded = sharded
    run.in_names = in_names
    run.zero_shapes = zero_shapes
    run.mesh = mesh
    return run


def _get_runner():
    global _RUNNER_CACHE
    if _RUNNER_CACHE is None:
        _RUNNER_CACHE = _make_runner(_get_nc())
    return _RUNNER_CACHE


def kernel(x, w_qkv, w_proj):
    in_maps = shard_inputs(x, w_qkv, w_proj)
    try:
        results = _get_runner()(in_maps)
    except Exception:
        res = run_bass_kernel_spmd(_get_nc(), in_maps, list(range(N_CORES)))
        results = res.results
    return assemble_output(results)
